# revision 7
# baseline (speedup 1.0000x reference)
"""ABMIL gated-attention bag classifier — Trainium2 Bass kernel.

Problem: B=16 bags x N=8192 instances x E=512 features, P=128 hidden, C=1.
  A_v = tanh(x @ Wv + bv); A_u = sigmoid(x @ Wu + bu)
  logits = (A_v * A_u) @ Wa + ba            [B, N, 1]
  A = softmax(mask(logits), axis=N)          (instances >= bag_len masked out)
  pooled = einsum('bnc,bne->bce', A, x)      [B, 1, 512]
Returns (A, pooled).

Sharding: data-parallel over bags — 8 cores x 2 bags each; the tiny weights
are replicated. Single pass over x per core (no max-subtraction needed for
the softmax: |logit| <= sum|Wa| + |ba| < 12, so exp() cannot overflow).

Per 256-token tile (32 tiles per bag):
  DMA x tile (fp32, contiguous HBM) -> PE transpose to get x^T (E on
  partitions, float32r mode) -> DVE evict to SBUF -> PE matmuls v,u
  (fp32r, full-rate since moving dim >= 256) -> ACT tanh/sigmoid (+bias)
  -> DVE gate multiply -> PE logit matmul (tokens on PSUM partitions)
  -> ACT exp with per-token bias = ba - 1000*(token >= bag_len), which
  masks via fp32 underflow -> PE pooling matmul accumulated over the
  whole bag in one PSUM bank.  Bag epilogue: Z = sum(p) via DVE free-dim
  reduce + GpSimd partition all-reduce, then normalize A and pooled.
"""

import os
import sys

import numpy as np

for _p in ("/opt/trn_rl_repo", "/root/.axon_site/_ro/trn_rl_repo"):
    if os.path.isdir(_p) and _p not in sys.path:
        sys.path.insert(0, _p)

import concourse.bass as bass
import concourse.bacc as bacc
import concourse.bass_isa as bass_isa
import concourse.mybir as mybir
import concourse.tile as tile
from concourse.masks import make_identity

F32 = mybir.dt.float32
F32R = mybir.dt.float32r
I32 = mybir.dt.int32
AF = mybir.ActivationFunctionType

B, N, E, P = 16, 8192, 512, 128
NCORES = 8
NB = B // NCORES          # bags per core
TILE_TOK = 256            # tokens per tile
NSUB = TILE_TOK // 128    # 128-token subtiles per tile
NTILES = N // TILE_TOK    # tiles per bag
NCOLS = N // 128          # subtile columns per bag (pAT free dim)
MASK_NEG = 1000.0         # exp(logit - ~1000) underflows fp32 to exactly 0


def build_nc():
    nc = bacc.Bacc("TRN2", target_bir_lowering=False, debug=False)

    x_d = nc.dram_tensor("x", [NB, N, E], F32R, kind="ExternalInput")
    lens_d = nc.dram_tensor("lens", [NB], I32, kind="ExternalInput")
    wv_d = nc.dram_tensor("Wv", [E, P], F32R, kind="ExternalInput")
    bv_d = nc.dram_tensor("bv", [P], F32, kind="ExternalInput")
    wu_d = nc.dram_tensor("Wu", [E, P], F32R, kind="ExternalInput")
    bu_d = nc.dram_tensor("bu", [P], F32, kind="ExternalInput")
    wa_d = nc.dram_tensor("Wa", [P, 1], F32, kind="ExternalInput")
    ba_d = nc.dram_tensor("ba", [1], F32, kind="ExternalInput")
    a_d = nc.dram_tensor("A_out", [NB, N], F32, kind="ExternalOutput")
    pooled_d = nc.dram_tensor("pooled_out", [NB, E], F32, kind="ExternalOutput")

    from contextlib import ExitStack

    with tile.TileContext(nc) as tc, ExitStack() as ctx:
        consts = ctx.enter_context(tc.tile_pool(name="consts", bufs=1))
        bagp = ctx.enter_context(tc.tile_pool(name="bagp", bufs=2))
        xtp = ctx.enter_context(tc.tile_pool(name="xtp", bufs=3))
        xtsp = ctx.enter_context(tc.tile_pool(name="xtsp", bufs=2))
        actp = ctx.enter_context(tc.tile_pool(name="actp", bufs=2))
        patp = ctx.enter_context(tc.tile_pool(name="patp", bufs=2))
        outp = ctx.enter_context(tc.tile_pool(name="outp", bufs=2))
        # PSUM budget (8 banks): xT 2x2 + v 1 + u 1 + logits 1 + pool 1 = 8
        xtps_pool = ctx.enter_context(tc.tile_pool(name="xtps", bufs=2, space="PSUM"))
        vps_pool = ctx.enter_context(tc.tile_pool(name="vps", bufs=1, space="PSUM"))
        ups_pool = ctx.enter_context(tc.tile_pool(name="ups", bufs=1, space="PSUM"))
        plps_pool = ctx.enter_context(tc.tile_pool(name="plps", bufs=1, space="PSUM"))
        poolps_pool = ctx.enter_context(
            tc.tile_pool(name="poolps", bufs=1, space="PSUM")
        )

        # ---- constants ----
        ident_f = consts.tile([128, 128], F32, tag="ident_f")
        make_identity(nc, ident_f)
        ident = consts.tile([128, 128], F32R, tag="ident")
        nc.vector.tensor_copy(out=ident, in_=ident_f)

        wv_sb = consts.tile([128, E // 128, P], F32R, tag="wv")
        nc.sync.dma_start(out=wv_sb, in_=wv_d[:].rearrange("(c k) p -> k c p", k=128))
        wu_sb = consts.tile([128, E // 128, P], F32R, tag="wu")
        nc.sync.dma_start(out=wu_sb, in_=wu_d[:].rearrange("(c k) p -> k c p", k=128))
        wa_sb = consts.tile([128, 1], F32, tag="wa")
        nc.sync.dma_start(out=wa_sb, in_=wa_d[:, :])
        bv_sb = consts.tile([128, 1], F32, tag="bv")
        nc.sync.dma_start(out=bv_sb, in_=bv_d[:][:, None])
        bu_sb = consts.tile([128, 1], F32, tag="bu")
        nc.sync.dma_start(out=bu_sb, in_=bu_d[:][:, None])

        lens_sb = consts.tile([1, NB], I32, tag="lens")
        nc.sync.dma_start(out=lens_sb, in_=lens_d[:][None, :])
        ba_sb = consts.tile([1, 1], F32, tag="ba")
        nc.sync.dma_start(out=ba_sb, in_=ba_d[:][None, :])
        ba_bc = consts.tile([128, 1], F32, tag="ba_bc")
        nc.gpsimd.partition_broadcast(ba_bc, ba_sb)
        ba_m = consts.tile([128, 1], F32, tag="ba_m")
        nc.vector.tensor_scalar_add(ba_m, ba_bc, -MASK_NEG)

        iota_i = consts.tile([128, NCOLS], I32, tag="iota_i")
        nc.gpsimd.iota(iota_i, pattern=[[128, NCOLS]], base=0, channel_multiplier=1)
        iota_f = consts.tile([128, NCOLS], F32, tag="iota_f")
        nc.vector.tensor_copy(out=iota_f, in_=iota_i)

        for b in range(NB):
            # ---- bag setup: mask bias column ba - 1000*(tok >= len) ----
            lenb_i = bagp.tile([128, 1], I32, tag="lenb_i")
            nc.gpsimd.partition_broadcast(lenb_i, lens_sb[0:1, b : b + 1])
            lenb_f = bagp.tile([128, 1], F32, tag="lenb_f")
            nc.vector.tensor_copy(out=lenb_f, in_=lenb_i)
            maskf = bagp.tile([128, NCOLS], F32, tag="maskf")
            nc.vector.tensor_scalar(
                out=maskf, in0=iota_f, scalar1=lenb_f, scalar2=None,
                op0=mybir.AluOpType.is_lt,
            )
            maskadd = bagp.tile([128, NCOLS], F32, tag="maskadd")
            nc.vector.tensor_scalar(
                out=maskadd, in0=maskf, scalar1=MASK_NEG, scalar2=ba_m,
                op0=mybir.AluOpType.mult, op1=mybir.AluOpType.add,
            )

            pat = patp.tile([128, NCOLS], F32R, tag="pat")
            pool_ps = poolps_pool.tile([1, E], F32, tag="pool_ps")

            for i in range(NTILES):
                t0 = i * TILE_TOK
                xt = xtp.tile([128, NSUB, E], F32R, tag="xt")
                nc.sync.dma_start(
                    out=xt,
                    in_=x_d[b, t0 : t0 + TILE_TOK, :].rearrange(
                        "(s k) e -> k s e", k=128
                    ),
                )
                # x^T: [E-part, token] via PE transpose-mode
                xt_ps = xtps_pool.tile([128, NSUB * E // 128 * 128], F32R, tag="xt_ps")
                for s in range(NSUB):
                    for c in range(E // 128):
                        o = c * (NSUB * 128) + s * 128
                        nc.tensor.transpose(
                            out=xt_ps[:, o : o + 128],
                            in_=xt[:, s, c * 128 : (c + 1) * 128],
                            identity=ident,
                        )
                xt_sb = xtsp.tile([128, NSUB * E], F32R, tag="xt_sb")
                nc.vector.tensor_copy(out=xt_sb, in_=xt_ps)

                v_ps = vps_pool.tile([128, TILE_TOK], F32, tag="v_ps")
                u_ps = ups_pool.tile([128, TILE_TOK], F32, tag="u_ps")
                for c in range(E // 128):
                    nc.tensor.matmul(
                        v_ps, lhsT=wv_sb[:, c, :],
                        rhs=xt_sb[:, c * TILE_TOK : (c + 1) * TILE_TOK],
                        start=(c == 0), stop=(c == E // 128 - 1),
                    )
                for c in range(E // 128):
                    nc.tensor.matmul(
                        u_ps, lhsT=wu_sb[:, c, :],
                        rhs=xt_sb[:, c * TILE_TOK : (c + 1) * TILE_TOK],
                        start=(c == 0), stop=(c == E // 128 - 1),
                    )
                av = actp.tile([128, TILE_TOK], F32, tag="av")
                nc.scalar.activation(out=av, in_=v_ps, func=AF.Tanh, bias=bv_sb)
                au = actp.tile([128, TILE_TOK], F32, tag="au")
                nc.scalar.activation(out=au, in_=u_ps, func=AF.Sigmoid, bias=bu_sb)
                g = actp.tile([128, TILE_TOK], F32, tag="g")
                nc.vector.tensor_mul(g, av, au)

                pl_ps = plps_pool.tile([128, NSUB], F32, tag="pl_ps")
                for s in range(NSUB):
                    nc.tensor.matmul(
                        pl_ps[:, s : s + 1],
                        lhsT=g[:, s * 128 : (s + 1) * 128],
                        rhs=wa_sb,
                        start=True, stop=True,
                    )
                for s in range(NSUB):
                    col = NSUB * i + s
                    nc.scalar.activation(
                        out=pat[:, col : col + 1], in_=pl_ps[:, s : s + 1],
                        func=AF.Exp, bias=maskadd[:, col : col + 1],
                    )
                for s in range(NSUB):
                    col = NSUB * i + s
                    nc.tensor.matmul(
                        pool_ps, lhsT=pat[:, col : col + 1],
                        rhs=xt[:, s, :],
                        start=(col == 0), stop=(col == NCOLS - 1),
                    )

            # ---- bag epilogue: Z, normalize, store ----
            rowsum = bagp.tile([128, 1], F32, tag="rowsum")
            nc.vector.reduce_sum(out=rowsum, in_=pat, axis=mybir.AxisListType.X)
            zall = bagp.tile([128, 1], F32, tag="zall")
            nc.gpsimd.partition_all_reduce(
                zall, rowsum, channels=128, reduce_op=bass_isa.ReduceOp.add
            )
            recipz = bagp.tile([128, 1], F32, tag="recipz")
            nc.vector.reciprocal(out=recipz, in_=zall)

            a_sb = outp.tile([128, NCOLS], F32, tag="a_sb")
            nc.vector.tensor_scalar_mul(a_sb, in0=pat, scalar1=recipz)
            nc.sync.dma_start(
                out=a_d[b, :].rearrange("(s k) -> k s", k=128), in_=a_sb
            )
            pooled_sb = outp.tile([1, E], F32, tag="pooled_sb")
            nc.vector.tensor_scalar_mul(
                pooled_sb, in0=pool_ps, scalar1=recipz[0:1, :]
            )
            nc.sync.dma_start(out=pooled_d[b : b + 1, :], in_=pooled_sb)

    nc.compile()
    return nc


LAST_RESULTS = None


def round_fp32r(a):
    """Round fp32 to the PE's fp32r storage format: 11-bit mantissa (RNE),
    low 12 bits of the word zero — matches walrus fp32_to_fp32r."""
    u = np.ascontiguousarray(a, dtype=np.float32).view(np.uint32)
    r = (u + 0x7FF + ((u >> 12) & 1)) & np.uint32(0xFFFFF000)
    return r.view(np.float32)


def make_in_maps(x, bag_lens, Wv, bv, Wu, bu, Wa, ba):
    x = round_fp32r(np.asarray(x, dtype=np.float32))
    lens = np.asarray(bag_lens).astype(np.int32)
    wv = round_fp32r(np.asarray(Wv, dtype=np.float32))
    bv = np.asarray(bv, dtype=np.float32)
    wu = round_fp32r(np.asarray(Wu, dtype=np.float32))
    bu = np.asarray(bu, dtype=np.float32)
    wa = np.asarray(Wa, dtype=np.float32)
    ba = np.asarray(ba, dtype=np.float32)
    in_maps = []
    for c in range(NCORES):
        in_maps.append({
            "x": np.ascontiguousarray(x[c * NB : (c + 1) * NB]),
            "lens": np.ascontiguousarray(lens[c * NB : (c + 1) * NB]),
            "Wv": wv, "bv": bv, "Wu": wu, "bu": bu, "Wa": wa, "ba": ba,
        })
    return in_maps


def kernel(x, bag_lens, Wv, bv, Wu, bu, Wa, ba):
    global LAST_RESULTS
    from concourse.bass_utils import run_bass_kernel_spmd

    nc = build_nc()
    in_maps = make_in_maps(x, bag_lens, Wv, bv, Wu, bu, Wa, ba)
    trace = bool(int(os.environ.get("ABMIL_TRACE", "0")))
    res = run_bass_kernel_spmd(
        nc, in_maps, core_ids=list(range(NCORES)), trace=trace
    )
    LAST_RESULTS = res
    A = np.empty((B, N, 1), dtype=np.float32)
    pooled = np.empty((B, 1, E), dtype=np.float32)
    for c in range(NCORES):
        A[c * NB : (c + 1) * NB, :, 0] = res.results[c]["A_out"]
        pooled[c * NB : (c + 1) * NB, 0, :] = res.results[c]["pooled_out"]
    return A, pooled


# revision 8
# speedup vs baseline: 1.9493x; 1.9493x over previous
"""ABMIL gated-attention bag classifier — Trainium2 Bass kernel.

Problem: B=16 bags x N=8192 instances x E=512 features, P=128 hidden, C=1.
  A_v = tanh(x @ Wv + bv); A_u = sigmoid(x @ Wu + bu)
  logits = (A_v * A_u) @ Wa + ba            [B, N, 1]
  A = softmax(mask(logits), axis=N)          (instances >= bag_len masked out)
  pooled = einsum('bnc,bne->bce', A, x)      [B, 1, 512]
Returns (A, pooled).

Sharding: data-parallel over bags — 8 cores x 2 bags each; the tiny weights
are replicated. Single pass over x per core (no max-subtraction needed for
the softmax: |logit| <= sum|Wa| + |ba| < 12, so exp() cannot overflow and
exp(logit)+masking-by-zero is exact).

v2 pipeline, per 256-token tile (32 tiles per bag):
  SWDGE DMA loads x fp32 from HBM casting to bf16 in SBUF -> PE
  transpose-mode (bf16, FWL) builds x^T in one bf16 PSUM bank -> DVE
  evicts to SBUF -> PE v,u matmuls (bf16, Wv/Wu stationary) -> ACT tanh
  for v and tanh(u/2) for u (sigmoid folded: sig(x)=(tanh(x/2)+1)/2, so
  only ONE activation table is resident) -> DVE affine+gate multiply ->
  PE logit matmul (g stationary, tokens on PSUM partitions) -> DVE evicts
  logits to SBUF.  Every GROUP tiles: one batched ACT exp + DVE masking
  (multiply by a 0/1 iota<len mask), producing fp32 p (for A and Z) and
  bf16 p (pooling lhsT); then the group's pooling matmuls accumulate
  p.T @ x into one fp32 PSUM bank for the whole bag.
  Bag epilogue: Z via DVE free-reduce + GpSimd partition all-reduce,
  reciprocal, normalize A and pooled, DMA out.
"""

import os
import sys

import numpy as np

for _p in ("/opt/trn_rl_repo", "/root/.axon_site/_ro/trn_rl_repo"):
    if os.path.isdir(_p) and _p not in sys.path:
        sys.path.insert(0, _p)

import concourse.bacc as bacc
import concourse.bass_isa as bass_isa
import concourse.mybir as mybir
import concourse.tile as tile
from concourse.masks import make_identity

F32 = mybir.dt.float32
BF16 = mybir.dt.bfloat16
I32 = mybir.dt.int32
AF = mybir.ActivationFunctionType

B, N, E, P = 16, 8192, 512, 128
NCORES = 8
NB = B // NCORES          # bags per core
TILE_TOK = 256            # tokens per tile
NSUB = TILE_TOK // 128    # 128-token subtiles per tile
NTILES = N // TILE_TOK    # tiles per bag
NCOLS = N // 128          # subtile columns per bag (pat free dim)
GROUP = 8                 # tiles per batched-exp group (pooling lags a group)
NGROUPS = NTILES // GROUP


def build_nc():
    nc = bacc.Bacc("TRN2", target_bir_lowering=False, debug=False)

    x_d = nc.dram_tensor("x", [NB, N, E], F32, kind="ExternalInput")
    lens_d = nc.dram_tensor("lens", [NB], I32, kind="ExternalInput")
    wv_d = nc.dram_tensor("Wv", [E, P], BF16, kind="ExternalInput")
    bv_d = nc.dram_tensor("bv", [P], F32, kind="ExternalInput")
    wu_d = nc.dram_tensor("Wu", [E, P], BF16, kind="ExternalInput")
    bu_d = nc.dram_tensor("bu", [P], F32, kind="ExternalInput")
    wa_d = nc.dram_tensor("Wa", [P, 1], BF16, kind="ExternalInput")
    ba_d = nc.dram_tensor("ba", [1], F32, kind="ExternalInput")
    a_d = nc.dram_tensor("A_out", [NB, N], F32, kind="ExternalOutput")
    pooled_d = nc.dram_tensor("pooled_out", [NB, E], F32, kind="ExternalOutput")

    from contextlib import ExitStack

    with tile.TileContext(nc) as tc, ExitStack() as ctx:
        consts = ctx.enter_context(tc.tile_pool(name="consts", bufs=1))
        bagp = ctx.enter_context(tc.tile_pool(name="bagp", bufs=2))
        xtp = ctx.enter_context(tc.tile_pool(name="xtp", bufs=GROUP + 3))
        xtsp = ctx.enter_context(tc.tile_pool(name="xtsp", bufs=2))
        actp = ctx.enter_context(tc.tile_pool(name="actp", bufs=2))
        patp = ctx.enter_context(tc.tile_pool(name="patp", bufs=2))
        outp = ctx.enter_context(tc.tile_pool(name="outp", bufs=2))
        # PSUM banks: xT 2x1 + v 1 + u 1 + logits 2 + pool 2 = 8
        xtps_pool = ctx.enter_context(tc.tile_pool(name="xtps", bufs=2, space="PSUM"))
        vps_pool = ctx.enter_context(tc.tile_pool(name="vps", bufs=1, space="PSUM"))
        ups_pool = ctx.enter_context(tc.tile_pool(name="ups", bufs=1, space="PSUM"))
        plps_pool = ctx.enter_context(tc.tile_pool(name="plps", bufs=2, space="PSUM"))
        poolps_pool = ctx.enter_context(
            tc.tile_pool(name="poolps", bufs=2, space="PSUM")
        )

        # ---- constants ----
        ident_f = consts.tile([128, 128], F32, tag="ident_f")
        make_identity(nc, ident_f)
        ident = consts.tile([128, 128], BF16, tag="ident")
        nc.vector.tensor_copy(out=ident, in_=ident_f)

        wv_sb = consts.tile([128, E // 128, P], BF16, tag="wv")
        nc.sync.dma_start(out=wv_sb, in_=wv_d[:].rearrange("(c k) p -> k c p", k=128))
        wu_sb = consts.tile([128, E // 128, P], BF16, tag="wu")
        nc.sync.dma_start(out=wu_sb, in_=wu_d[:].rearrange("(c k) p -> k c p", k=128))
        wa_sb = consts.tile([128, 1], BF16, tag="wa")
        nc.sync.dma_start(out=wa_sb, in_=wa_d[:, :])
        bv_sb = consts.tile([128, 1], F32, tag="bv")
        nc.sync.dma_start(out=bv_sb, in_=bv_d[:][:, None])
        bu_sb = consts.tile([128, 1], F32, tag="bu")
        nc.sync.dma_start(out=bu_sb, in_=bu_d[:][:, None])
        # tanh-fold for sigmoid: sig(u) = (tanh(0.5*u + 0.5*bu) + 1) / 2
        buh_sb = consts.tile([128, 1], F32, tag="buh")
        nc.vector.tensor_scalar_mul(buh_sb, bu_sb, 0.5)

        lens_sb = consts.tile([1, NB], I32, tag="lens")
        nc.sync.dma_start(out=lens_sb, in_=lens_d[:][None, :])
        ba_sb = consts.tile([1, 1], F32, tag="ba")
        nc.sync.dma_start(out=ba_sb, in_=ba_d[:][None, :])
        ba_bc = consts.tile([128, 1], F32, tag="ba_bc")
        nc.gpsimd.partition_broadcast(ba_bc, ba_sb)

        iota_i = consts.tile([128, NCOLS], I32, tag="iota_i")
        nc.gpsimd.iota(iota_i, pattern=[[128, NCOLS]], base=0, channel_multiplier=1)
        iota_f = consts.tile([128, NCOLS], F32, tag="iota_f")
        nc.vector.tensor_copy(out=iota_f, in_=iota_i)

        for b in range(NB):
            # ---- bag setup: 0/1 valid mask per token column ----
            lenb_i = bagp.tile([128, 1], I32, tag="lenb_i")
            nc.gpsimd.partition_broadcast(lenb_i, lens_sb[0:1, b : b + 1])
            lenb_f = bagp.tile([128, 1], F32, tag="lenb_f")
            nc.vector.tensor_copy(out=lenb_f, in_=lenb_i)
            maskf = bagp.tile([128, NCOLS], F32, tag="maskf")
            nc.vector.tensor_scalar(
                out=maskf, in0=iota_f, scalar1=lenb_f, scalar2=None,
                op0=mybir.AluOpType.is_lt,
            )

            pat_f = patp.tile([128, NCOLS], F32, tag="pat_f")
            pat_b = patp.tile([128, NCOLS], BF16, tag="pat_b")
            pl_sb = patp.tile([128, NCOLS], F32, tag="pl_sb")
            pool_ps = poolps_pool.tile([1, E], F32, tag="pool_ps")

            xts = {}
            for gi in range(NGROUPS):
                for i in range(gi * GROUP, (gi + 1) * GROUP):
                    t0 = i * TILE_TOK
                    xt = xtp.tile([128, NSUB, E], BF16, tag="xt")
                    xts[i] = xt
                    nc.gpsimd.dma_start(
                        out=xt,
                        in_=x_d[b, t0 : t0 + TILE_TOK, :].rearrange(
                            "(s k) e -> k s e", k=128
                        ),
                    )
                    # x^T: [E-part, token] via PE transpose-mode (bf16)
                    xt_ps = xtps_pool.tile([128, NSUB * E], BF16, tag="xt_ps")
                    for s in range(NSUB):
                        for c in range(E // 128):
                            o = c * (NSUB * 128) + s * 128
                            nc.tensor.transpose(
                                out=xt_ps[:, o : o + 128],
                                in_=xt[:, s, c * 128 : (c + 1) * 128],
                                identity=ident,
                            )
                    xt_sb = xtsp.tile([128, NSUB * E], BF16, tag="xt_sb")
                    nc.vector.tensor_copy(out=xt_sb, in_=xt_ps)

                    v_ps = vps_pool.tile([128, TILE_TOK], F32, tag="v_ps")
                    u_ps = ups_pool.tile([128, TILE_TOK], F32, tag="u_ps")
                    for c in range(E // 128):
                        nc.tensor.matmul(
                            v_ps, lhsT=wv_sb[:, c, :],
                            rhs=xt_sb[:, c * TILE_TOK : (c + 1) * TILE_TOK],
                            start=(c == 0), stop=(c == E // 128 - 1),
                        )
                    for c in range(E // 128):
                        nc.tensor.matmul(
                            u_ps, lhsT=wu_sb[:, c, :],
                            rhs=xt_sb[:, c * TILE_TOK : (c + 1) * TILE_TOK],
                            start=(c == 0), stop=(c == E // 128 - 1),
                        )
                    av = actp.tile([128, TILE_TOK], BF16, tag="av")
                    nc.scalar.activation(out=av, in_=v_ps, func=AF.Tanh, bias=bv_sb)
                    tu = actp.tile([128, TILE_TOK], BF16, tag="tu")
                    nc.scalar.activation(
                        out=tu, in_=u_ps, func=AF.Tanh, bias=buh_sb, scale=0.5
                    )
                    au = actp.tile([128, TILE_TOK], BF16, tag="au")
                    nc.vector.tensor_scalar(
                        out=au, in0=tu, scalar1=1.0, scalar2=0.5,
                        op0=mybir.AluOpType.add, op1=mybir.AluOpType.mult,
                    )
                    g = actp.tile([128, TILE_TOK], BF16, tag="g")
                    nc.vector.tensor_mul(g, av, au)

                    pl_ps = plps_pool.tile([128, NSUB], F32, tag="pl_ps")
                    for s in range(NSUB):
                        nc.tensor.matmul(
                            pl_ps[:, s : s + 1],
                            lhsT=g[:, s * 128 : (s + 1) * 128], rhs=wa_sb,
                            start=True, stop=True,
                        )
                    col = NSUB * i
                    nc.vector.tensor_copy(
                        out=pl_sb[:, col : col + NSUB], in_=pl_ps
                    )

                # ---- batched exp + masking for the whole group ----
                c0 = gi * GROUP * NSUB
                c1 = (gi + 1) * GROUP * NSUB
                et = bagp.tile([128, GROUP * NSUB], F32, tag="et")
                nc.scalar.activation(
                    out=et, in_=pl_sb[:, c0:c1], func=AF.Exp, bias=ba_bc
                )
                nc.vector.tensor_mul(pat_f[:, c0:c1], et, maskf[:, c0:c1])
                nc.vector.tensor_mul(pat_b[:, c0:c1], et, maskf[:, c0:c1])

                # ---- pooling matmuls for the group ----
                for i in range(gi * GROUP, (gi + 1) * GROUP):
                    for s in range(NSUB):
                        col = NSUB * i + s
                        nc.tensor.matmul(
                            pool_ps, lhsT=pat_b[:, col : col + 1],
                            rhs=xts[i][:, s, :],
                            start=(col == 0), stop=(col == NCOLS - 1),
                        )

            # ---- bag epilogue: Z, normalize, store ----
            rowsum = bagp.tile([128, 1], F32, tag="rowsum")
            nc.vector.reduce_sum(out=rowsum, in_=pat_f, axis=mybir.AxisListType.X)
            zall = bagp.tile([128, 1], F32, tag="zall")
            nc.gpsimd.partition_all_reduce(
                zall, rowsum, channels=128, reduce_op=bass_isa.ReduceOp.add
            )
            recipz = bagp.tile([128, 1], F32, tag="recipz")
            nc.vector.reciprocal(out=recipz, in_=zall)

            a_sb = outp.tile([128, NCOLS], F32, tag="a_sb")
            nc.vector.tensor_scalar_mul(a_sb, in0=pat_f, scalar1=recipz)
            nc.sync.dma_start(
                out=a_d[b, :].rearrange("(s k) -> k s", k=128), in_=a_sb
            )
            pooled_sb = outp.tile([1, E], F32, tag="pooled_sb")
            nc.vector.tensor_scalar_mul(
                pooled_sb, in0=pool_ps, scalar1=recipz[0:1, :]
            )
            nc.sync.dma_start(out=pooled_d[b : b + 1, :], in_=pooled_sb)

    nc.compile()
    return nc


LAST_RESULTS = None


def make_in_maps(x, bag_lens, Wv, bv, Wu, bu, Wa, ba):
    import ml_dtypes

    x = np.ascontiguousarray(np.asarray(x, dtype=np.float32))
    lens = np.asarray(bag_lens).astype(np.int32)
    wv = np.asarray(Wv, dtype=np.float32).astype(ml_dtypes.bfloat16)
    bv = np.asarray(bv, dtype=np.float32)
    wu = np.asarray(Wu, dtype=np.float32).astype(ml_dtypes.bfloat16)
    bu = np.asarray(bu, dtype=np.float32)
    wa = np.asarray(Wa, dtype=np.float32).astype(ml_dtypes.bfloat16)
    ba = np.asarray(ba, dtype=np.float32)
    in_maps = []
    for c in range(NCORES):
        in_maps.append({
            "x": np.ascontiguousarray(x[c * NB : (c + 1) * NB]),
            "lens": np.ascontiguousarray(lens[c * NB : (c + 1) * NB]),
            "Wv": wv, "bv": bv, "Wu": wu, "bu": bu, "Wa": wa, "ba": ba,
        })
    return in_maps


def kernel(x, bag_lens, Wv, bv, Wu, bu, Wa, ba):
    global LAST_RESULTS
    from concourse.bass_utils import run_bass_kernel_spmd

    nc = build_nc()
    in_maps = make_in_maps(x, bag_lens, Wv, bv, Wu, bu, Wa, ba)
    trace = bool(int(os.environ.get("ABMIL_TRACE", "0")))
    res = run_bass_kernel_spmd(
        nc, in_maps, core_ids=list(range(NCORES)), trace=trace
    )
    LAST_RESULTS = res
    A = np.empty((B, N, 1), dtype=np.float32)
    pooled = np.empty((B, 1, E), dtype=np.float32)
    for c in range(NCORES):
        A[c * NB : (c + 1) * NB, :, 0] = res.results[c]["A_out"]
        pooled[c * NB : (c + 1) * NB, 0, :] = res.results[c]["pooled_out"]
    return A, pooled
